# revision 12
# baseline (speedup 1.0000x reference)
"""Multi-head attention (B=4,S=2048,D=1024,H=16,dh=65) on 8 TRN2 NeuronCores.

Sharding: batch x head-half. Core c handles batch c//2 and heads
(c%2)*8..(c%2)*8+8 (P-slice of 520). Each core computes its QKV projections,
attention, and a partial out-projection; the host sums the two partials per
batch and adds bo.

Compute dtype bf16 (fp32 PSUM accumulation); softmax runs unnormalized
(no max subtraction -- score magnitudes are bounded ~20 so exp stays in fp32
range) with the row-sum harvested from a leading ones-column in V.
"""

import math
import sys

import numpy as np
import ml_dtypes

sys.path.insert(0, "/opt/trn_rl_repo")

import concourse.bass as bass
import concourse.mybir as mybir
import concourse.tile as tile_mod
from concourse.bass_utils import run_bass_kernel_spmd
from concourse.masks import make_identity
from concourse.vector_clock import ScopedClock

# ---------------------------------------------------------------------------
# Patch for this container's walrus build: it rejects instructions carrying
# more than one semaphore wait ("Too many sync wait commands"), but Tile's
# wait assigner freely attaches several. Split excess waits onto bass_nofuse
# InstNoOp carriers on the same engine, committed immediately before the
# instruction (same-engine program order => over-synchronization only).
# ---------------------------------------------------------------------------
_MAX_WAITS = 1

_orig_commit = tile_mod.TileContext._commit_instruction


def _split_waits(self, inst, commit):
    si = inst.sync_info
    if si is None or len(si.on_wait) <= _MAX_WAITS:
        return
    waits = list(si.on_wait)
    sem_w = [w for w in waits if getattr(w, "sync_type", "semaphore") == "semaphore"]
    other_w = [w for w in waits if getattr(w, "sync_type", "semaphore") != "semaphore"]
    keep_budget = _MAX_WAITS - len(other_w)
    if keep_budget < 0:
        return
    keep = other_w + (sem_w[-keep_budget:] if keep_budget > 0 else [])
    excess = sem_w[: len(sem_w) - max(keep_budget, 0)]
    if not excess:
        return
    for i, w in enumerate(excess):
        nop = mybir.InstNoOp(
            name=f"{inst.name}-sw{i}",
            sync_info=mybir.SyncInfo(on_wait=[w], on_update=[]),
            bass_nofuse=True,
            engine=inst.engine,
        )
        commit(nop)
    inst.sync_info = mybir.SyncInfo(on_wait=keep, on_update=list(si.on_update))


def _patched_commit(self, inst, lazy_reg_writes: bool = True):
    if inst.engine != mybir.EngineType.Unassigned:
        _split_waits(self, inst, lambda n: _orig_commit(self, n, False))
    return _orig_commit(self, inst, lazy_reg_writes)


def _patched_drain_and_barrier(self, tick_clock, wait_clock):
    drain_inst = self.nc.sync.drain()
    wait_clock.add_sem_waits(
        drain_inst.ins, ScopedClock({None: tick_clock.global_clock})
    )
    si = drain_inst.ins.sync_info
    if si is not None and len(si.on_wait) > _MAX_WAITS:
        waits = list(si.on_wait)
        drain_inst.ins.sync_info = mybir.SyncInfo(
            on_wait=waits[:_MAX_WAITS], on_update=list(si.on_update)
        )
        for w in waits[_MAX_WAITS:]:
            n = self.nc.sync.nop(nofuse=True)
            n.ins.sync_info = mybir.SyncInfo(on_wait=[w], on_update=[])
    self.nc.all_engine_barrier()
    popped = self.nc._tile_sem_poison_stack.pop()
    assert popped is self._sem_poison
    self.nc.clear_and_free_semaphores(list(self.sems.allocated().values()))
    self.nc.all_engine_barrier()


tile_mod.TileContext._commit_instruction = _patched_commit
tile_mod.TileContext._drain_and_barrier = _patched_drain_and_barrier

# ---------------------------------------------------------------------------

B, S, D, H = 4, 2048, 1024, 16
DH = D // H + 1          # 65
P = H * DH               # 1040
HPC = H // 2             # heads per core
PC = HPC * DH            # 520, per-core P slice
N_CORES = 8

MT = S // 128            # 16 row blocks / k tiles
KT = 16                  # k tiles per attention
QB = 4                   # q blocks of 512
QW = 512
RKT = 2                  # k-tiles per score round (2 banks, double-buffered)
NR = KT // RKT           # 8 rounds

F32 = mybir.dt.float32
BF16 = mybir.dt.bfloat16
BF = ml_dtypes.bfloat16

_BUILT = {}


def _build_nc():
    nc = bass.Bass("TRN2", target_bir_lowering=False, debug=False,
                   num_devices=N_CORES)

    xq = nc.dram_tensor("xq", [D, S], BF16, kind="ExternalInput").ap()
    xk = nc.dram_tensor("xk", [D, S], BF16, kind="ExternalInput").ap()
    xv = nc.dram_tensor("xv", [D, S], BF16, kind="ExternalInput").ap()
    # maskH[qb, p, j*QW+q] = maskT[j*128+p, qb*512+q] (multiplicative 0/1)
    mh = nc.dram_tensor("maskH", [QB, 128, KT * QW], BF16,
                        kind="ExternalInput").ap()
    wq = nc.dram_tensor("wqT", [D, PC], BF16, kind="ExternalInput").ap()
    wk = nc.dram_tensor("wkT", [D, PC], BF16, kind="ExternalInput").ap()
    wv = nc.dram_tensor("wvT", [D, PC], BF16, kind="ExternalInput").ap()
    bq = nc.dram_tensor("bq", [1, PC], BF16, kind="ExternalInput").ap()
    bk = nc.dram_tensor("bk", [1, PC], BF16, kind="ExternalInput").ap()
    bv = nc.dram_tensor("bv", [1, PC], BF16, kind="ExternalInput").ap()
    wo = nc.dram_tensor("woT", [PC, D], BF16, kind="ExternalInput").ap()
    sel8_d = nc.dram_tensor("sel8", [HPC, HPC * DH], F32,
                            kind="ExternalInput").ap()
    out = nc.dram_tensor("out", [S, D], F32, kind="ExternalOutput").ap()

    with tile_mod.TileContext(nc) as tc:
        with tc.tile_pool(name="const", bufs=1) as pconst, \
             tc.tile_pool(name="qkT", bufs=1) as pqkT, \
             tc.tile_pool(name="vh", bufs=MT + 1) as pvh, \
             tc.tile_pool(name="psS", bufs=2, space="PSUM") as psS, \
             tc.tile_pool(name="psA", bufs=4, space="PSUM") as psA:

            ident = pconst.tile([128, 128], BF16, tag="ident")
            make_identity(nc, ident[:])
            ones_col = pconst.tile([1, 128], BF16, tag="ones")
            nc.gpsimd.memset(ones_col[:], 1.0)
            # sel8[k, h*DH+j] = (k == h): PE row-broadcast selector that
            # copies head h's reciprocal row onto partitions 0..64.
            sel8 = pconst.tile([HPC, HPC * DH], F32, tag="sel8")
            nc.sync.dma_start(sel8[:], sel8_d[:])

            # [65, proj(q=0,k=1), head, S]
            qkT = pqkT.tile([DH, 2, HPC, S], BF16, tag="qkT")
            # v k-tiles with trailing ones column: [128, head, 65+1]
            vh = [pvh.tile([128, HPC, DH + 1], BF16, tag="vh", name=f"vh{j}")
                  for j in range(MT)]
            for j in range(MT):
                nc.gpsimd.memset(vh[j][:, :, DH:DH + 1], 1.0)

            # ---------------- phase 1: projections --------------------------
            with tc.tile_pool(name="px", bufs=9) as px, \
                 tc.tile_pool(name="pw", bufs=9) as pw, \
                 tc.tile_pool(name="pb", bufs=3) as pb, \
                 tc.tile_pool(name="pt1", bufs=6) as pt1:
                for pi, (xd, wd, bd) in [(1, (xk, wk, bk)),
                                         (0, (xq, wq, bq)),
                                         (2, (xv, wv, bv))]:
                    xts, wts = [], []
                    for d in range(8):
                        xt = px.tile([128, S], BF16, tag="x")
                        nc.sync.dma_start(xt[:], xd[d * 128:(d + 1) * 128, :])
                        xts.append(xt)
                        wt = pw.tile([128, HPC, DH], BF16, tag="w")
                        nc.sync.dma_start(wt[:], wd[d * 128:(d + 1) * 128, :])
                        wts.append(wt)
                    bt = pb.tile([1, HPC, DH], BF16, tag="b")
                    nc.sync.dma_start(bt[:], bd[:])

                    for m in range(16):
                        for half in range(2):
                            hs = half * 4
                            ps = psA.tile([128, 4, DH], F32, tag="psA")
                            nc.tensor.matmul(
                                ps[:], ones_col[0:1, :],
                                bt[0:1, hs:hs + 4, :],
                                start=True, stop=False)
                            for d in range(8):
                                nc.tensor.matmul(
                                    ps[:],
                                    xts[d][:, m * 128:(m + 1) * 128],
                                    wts[d][:, hs:hs + 4, :],
                                    start=False, stop=(d == 7))
                            if pi == 2:  # v: straight copy into vh k-tiles
                                nc.vector.tensor_copy(
                                    vh[m][:, hs:hs + 4, 0:DH], ps[:])
                            else:       # q/k: bf16 copy, then PE transpose
                                row = pt1.tile([128, 4, DH], BF16, tag="row")
                                nc.vector.tensor_copy(row[:], ps[:])
                                pstr = psA.tile([128, 4, 128], BF16, tag="psA")
                                for t in range(4):
                                    nc.tensor.transpose(
                                        pstr[0:DH, t, :], row[:, t, :],
                                        ident[:])
                                nc.scalar.copy(
                                    qkT[0:DH, pi, hs:hs + 4,
                                        m * 128:(m + 1) * 128],
                                    pstr[0:DH, :, :])

            # ---------------- phase 2+3: attention + out-proj ---------------
            with tc.tile_pool(name="pm", bufs=1) as pm, \
                 tc.tile_pool(name="pp", bufs=4) as pp, \
                 tc.tile_pool(name="pc", bufs=1) as pc, \
                 tc.tile_pool(name="pwo", bufs=1) as pwo, \
                 tc.tile_pool(name="po", bufs=3) as po, \
                 tc.tile_pool(name="pt2", bufs=4) as pt2:

                # concatT: [dh, head, S]
                ccT = pc.tile([DH, HPC, S], BF16, tag="ccT")
                wots = []
                for h in range(HPC):
                    wot = pwo.tile([DH, D], BF16, tag=f"wo{h}", name=f"wo{h}")
                    nc.sync.dma_start(wot[:], wo[h * DH:(h + 1) * DH, :])
                    wots.append(wot)

                inv_sqrt = 1.0 / math.sqrt(float(DH))
                for qb in range(QB):
                    mt = pm.tile([128, KT, QW], BF16, tag="mask")
                    nc.sync.dma_start(mt[:], mh[qb, :, :])
                    rsall = pt2.tile([HPC, QW], F32, tag="rsall",
                                     name=f"rsall{qb}")
                    uovs = []
                    for h in range(HPC):
                        # outT accumulator [d(65)+rowsum(1), q=512], one bank
                        ov = psA.tile([128, QW], F32, tag="psA",
                                      name=f"ov{qb}_{h}")
                        for r in range(NR):
                            ss = psS.tile([128, RKT, QW], F32, tag="psS")
                            for jj in range(RKT):
                                j = r * RKT + jj
                                nc.tensor.matmul(
                                    ss[:, jj, :],
                                    qkT[0:DH, 1, h, j * 128:(j + 1) * 128],
                                    qkT[0:DH, 0, h, qb * QW:(qb + 1) * QW],
                                    start=True, stop=True)
                            pt = pp.tile([128, RKT, QW], BF16, tag="pT")
                            nc.scalar.activation(
                                pt[:], ss[:],
                                mybir.ActivationFunctionType.Exp,
                                scale=inv_sqrt)
                            nc.vector.tensor_mul(
                                pt[:], pt[:],
                                mt[:, r * RKT:(r + 1) * RKT, :])
                            for jj in range(RKT):
                                j = r * RKT + jj
                                nc.tensor.matmul(
                                    ov[0:DH + 1, :],
                                    vh[j][:, h, :],
                                    pt[:, jj, :],
                                    start=(j == 0), stop=(j == KT - 1))
                        # stash unnormalized outT; ship rowsum row to rsall
                        uov = pt2.tile([DH, QW], BF16, tag="uov",
                                       name=f"uov{qb}_{h}", bufs=HPC + 2)
                        nc.vector.tensor_copy(uov[:], ov[0:DH, :])
                        rs2 = pt2.tile([66, QW], F32, tag="rs2",
                                       name=f"rs2_{qb}_{h}")
                        nc.vector.tensor_copy(rs2[64:66, :], ov[64:66, :])
                        nc.gpsimd.dma_start(rsall[h:h + 1, :], rs2[65:66, :])
                        uovs.append(uov)
                    # one exact reciprocal for all 8 heads of this q-block,
                    # then per head: PE-broadcast onto 65 partitions + scale.
                    rcall = pt2.tile([HPC, QW], F32, tag="rcall",
                                     name=f"rcall{qb}")
                    nc.vector.reciprocal(rcall[:], rsall[:])
                    for h in range(HPC):
                        rbp = psA.tile([128, QW], F32, tag="psA",
                                       name=f"rbp{qb}_{h}")
                        nc.tensor.matmul(rbp[0:DH, :],
                                         sel8[:, h * DH:(h + 1) * DH],
                                         rcall[:], start=True, stop=True)
                        nc.vector.tensor_mul(
                            ccT[0:DH, h, qb * QW:(qb + 1) * QW],
                            rbp[0:DH, :], uovs[h][:])
                    # out-projection for this q-block's 4 row blocks --
                    # overlaps the next q-block's ACT/DVE-bound attention
                    for m in range(qb * 4, qb * 4 + 4):
                        osb = po.tile([128, D], F32, tag="osb")
                        for n in range(2):
                            ps = psA.tile([128, QW], F32, tag="psA",
                                          name=f"psop{m}_{n}")
                            for h in range(HPC):
                                nc.tensor.matmul(
                                    ps[:],
                                    ccT[0:DH, h, m * 128:(m + 1) * 128],
                                    wots[h][:, n * QW:(n + 1) * QW],
                                    start=(h == 0), stop=(h == HPC - 1))
                            nc.vector.tensor_copy(
                                osb[:, n * QW:(n + 1) * QW], ps[:])
                        nc.gpsimd.dma_start(out[m * 128:(m + 1) * 128, :],
                                            osb[:])

    return nc


def _prep_inputs(q, k, v, mask, Wq, bqv, Wk, bkv, Wv, bvv, Wo):
    """Per-core input maps (numpy, host-side shard + cast)."""
    in_maps = []
    sel8 = np.zeros((HPC, HPC * DH), np.float32)
    for h in range(HPC):
        sel8[h, h * DH:(h + 1) * DH] = 1.0
    mask_h = {}
    for b in range(B):
        mt = (mask[b, 0] != 0).astype(np.float32).T  # [k, q]
        m4 = mt.reshape(KT, 128, QB, QW).transpose(2, 1, 0, 3)
        mask_h[b] = np.ascontiguousarray(m4.reshape(QB, 128, KT * QW)).astype(BF)
    for c in range(N_CORES):
        b, hh = c // 2, c % 2
        sl = slice(hh * PC, (hh + 1) * PC)
        in_maps.append({
            "xq": np.ascontiguousarray(q[b].T).astype(BF),
            "xk": np.ascontiguousarray(k[b].T).astype(BF),
            "xv": np.ascontiguousarray(v[b].T).astype(BF),
            "maskH": mask_h[b],
            "wqT": np.ascontiguousarray(Wq[sl, :].T).astype(BF),
            "wkT": np.ascontiguousarray(Wk[sl, :].T).astype(BF),
            "wvT": np.ascontiguousarray(Wv[sl, :].T).astype(BF),
            "bq": bqv[sl].reshape(1, PC).astype(BF),
            "bk": bkv[sl].reshape(1, PC).astype(BF),
            "bv": bvv[sl].reshape(1, PC).astype(BF),
            "woT": np.ascontiguousarray(Wo[:, sl].T).astype(BF),
            "sel8": sel8,
        })
    return in_maps


def run_sharded(in_maps, **kwargs):
    if "nc" not in _BUILT:
        _BUILT["nc"] = _build_nc()
    return run_bass_kernel_spmd(_BUILT["nc"], in_maps,
                                core_ids=list(range(N_CORES)), **kwargs)


def kernel(q, k, v, mask, Wq, bq, Wk, bk, Wv, bv, Wo, bo):
    q = np.asarray(q, np.float32)
    k = np.asarray(k, np.float32)
    v = np.asarray(v, np.float32)
    mask = np.asarray(mask)
    in_maps = _prep_inputs(q, k, v, mask,
                           np.asarray(Wq, np.float32), np.asarray(bq, np.float32),
                           np.asarray(Wk, np.float32), np.asarray(bk, np.float32),
                           np.asarray(Wv, np.float32), np.asarray(bv, np.float32),
                           np.asarray(Wo, np.float32))
    res = run_sharded(in_maps)
    bo32 = np.asarray(bo, np.float32)
    out = np.empty((B, S, D), np.float32)
    for b in range(B):
        out[b] = res.results[2 * b]["out"] + res.results[2 * b + 1]["out"] + bo32
    return out


# revision 14
# speedup vs baseline: 1.0121x; 1.0121x over previous
"""Multi-head attention (B=4,S=2048,D=1024,H=16,dh=65) on 8 TRN2 NeuronCores.

Sharding: batch x head-half. Core c handles batch c//2 and heads
(c%2)*8..(c%2)*8+8 (P-slice of 520). Each core computes its QKV projections,
attention, and a partial out-projection; the host sums the two partials per
batch and adds bo.

Compute dtype bf16 (fp32 PSUM accumulation); softmax runs unnormalized
(no max subtraction -- score magnitudes are bounded ~20 so exp stays in fp32
range) with the row-sum harvested from a leading ones-column in V.
"""

import math
import sys

import numpy as np
import ml_dtypes

sys.path.insert(0, "/opt/trn_rl_repo")

import concourse.bass as bass
import concourse.mybir as mybir
import concourse.tile as tile_mod
from concourse.bass_utils import run_bass_kernel_spmd
from concourse.masks import make_identity
from concourse.vector_clock import ScopedClock

# ---------------------------------------------------------------------------
# Patch for this container's walrus build: it rejects instructions carrying
# more than one semaphore wait ("Too many sync wait commands"), but Tile's
# wait assigner freely attaches several. Split excess waits onto bass_nofuse
# InstNoOp carriers on the same engine, committed immediately before the
# instruction (same-engine program order => over-synchronization only).
# ---------------------------------------------------------------------------
_MAX_WAITS = 1

_orig_commit = tile_mod.TileContext._commit_instruction


def _split_waits(self, inst, commit):
    si = inst.sync_info
    if si is None or len(si.on_wait) <= _MAX_WAITS:
        return
    waits = list(si.on_wait)
    sem_w = [w for w in waits if getattr(w, "sync_type", "semaphore") == "semaphore"]
    other_w = [w for w in waits if getattr(w, "sync_type", "semaphore") != "semaphore"]
    keep_budget = _MAX_WAITS - len(other_w)
    if keep_budget < 0:
        return
    keep = other_w + (sem_w[-keep_budget:] if keep_budget > 0 else [])
    excess = sem_w[: len(sem_w) - max(keep_budget, 0)]
    if not excess:
        return
    for i, w in enumerate(excess):
        nop = mybir.InstNoOp(
            name=f"{inst.name}-sw{i}",
            sync_info=mybir.SyncInfo(on_wait=[w], on_update=[]),
            bass_nofuse=True,
            engine=inst.engine,
        )
        commit(nop)
    inst.sync_info = mybir.SyncInfo(on_wait=keep, on_update=list(si.on_update))


def _patched_commit(self, inst, lazy_reg_writes: bool = True):
    if inst.engine != mybir.EngineType.Unassigned:
        _split_waits(self, inst, lambda n: _orig_commit(self, n, False))
    return _orig_commit(self, inst, lazy_reg_writes)


def _patched_drain_and_barrier(self, tick_clock, wait_clock):
    drain_inst = self.nc.sync.drain()
    wait_clock.add_sem_waits(
        drain_inst.ins, ScopedClock({None: tick_clock.global_clock})
    )
    si = drain_inst.ins.sync_info
    if si is not None and len(si.on_wait) > _MAX_WAITS:
        waits = list(si.on_wait)
        drain_inst.ins.sync_info = mybir.SyncInfo(
            on_wait=waits[:_MAX_WAITS], on_update=list(si.on_update)
        )
        for w in waits[_MAX_WAITS:]:
            n = self.nc.sync.nop(nofuse=True)
            n.ins.sync_info = mybir.SyncInfo(on_wait=[w], on_update=[])
    self.nc.all_engine_barrier()
    popped = self.nc._tile_sem_poison_stack.pop()
    assert popped is self._sem_poison
    self.nc.clear_and_free_semaphores(list(self.sems.allocated().values()))
    self.nc.all_engine_barrier()


tile_mod.TileContext._commit_instruction = _patched_commit
tile_mod.TileContext._drain_and_barrier = _patched_drain_and_barrier

# ---------------------------------------------------------------------------

B, S, D, H = 4, 2048, 1024, 16
DH = D // H + 1          # 65
P = H * DH               # 1040
HPC = H // 2             # heads per core
PC = HPC * DH            # 520, per-core P slice
N_CORES = 8

MT = S // 128            # 16 row blocks / k tiles
KT = 16                  # k tiles per attention
QB = 4                   # q blocks of 512
QW = 512
RKT = 2                  # k-tiles per score round (2 banks, double-buffered)
NR = KT // RKT           # 8 rounds

F32 = mybir.dt.float32
BF16 = mybir.dt.bfloat16
BF = ml_dtypes.bfloat16

_BUILT = {}


def _build_nc():
    nc = bass.Bass("TRN2", target_bir_lowering=False, debug=False,
                   num_devices=N_CORES)

    xq = nc.dram_tensor("xq", [D, S], BF16, kind="ExternalInput").ap()
    xk = nc.dram_tensor("xk", [D, S], BF16, kind="ExternalInput").ap()
    xv = nc.dram_tensor("xv", [D, S], BF16, kind="ExternalInput").ap()
    # maskH[qb, p, j*QW+q] = maskT[j*128+p, qb*512+q] (multiplicative 0/1)
    mh = nc.dram_tensor("maskH", [QB, 128, KT * QW], BF16,
                        kind="ExternalInput").ap()
    wq = nc.dram_tensor("wqT", [D, PC], BF16, kind="ExternalInput").ap()
    wk = nc.dram_tensor("wkT", [D, PC], BF16, kind="ExternalInput").ap()
    wv = nc.dram_tensor("wvT", [D, PC], BF16, kind="ExternalInput").ap()
    bq = nc.dram_tensor("bq", [1, PC], BF16, kind="ExternalInput").ap()
    bk = nc.dram_tensor("bk", [1, PC], BF16, kind="ExternalInput").ap()
    bv = nc.dram_tensor("bv", [1, PC], BF16, kind="ExternalInput").ap()
    wo = nc.dram_tensor("woT", [PC, D], BF16, kind="ExternalInput").ap()
    sel8_d = nc.dram_tensor("sel8", [HPC, HPC * DH], F32,
                            kind="ExternalInput").ap()
    out = nc.dram_tensor("out", [S, D], F32, kind="ExternalOutput").ap()

    with tile_mod.TileContext(nc) as tc:
        with tc.tile_pool(name="const", bufs=1) as pconst, \
             tc.tile_pool(name="qkT", bufs=1) as pqkT, \
             tc.tile_pool(name="vh", bufs=MT + 1) as pvh, \
             tc.tile_pool(name="psS", bufs=2, space="PSUM") as psS, \
             tc.tile_pool(name="psA", bufs=4, space="PSUM") as psA:

            ident = pconst.tile([128, 128], BF16, tag="ident")
            make_identity(nc, ident[:])
            ones_col = pconst.tile([1, 128], BF16, tag="ones")
            nc.gpsimd.memset(ones_col[:], 1.0)
            # sel8[k, h*DH+j] = (k == h): PE row-broadcast selector that
            # copies head h's reciprocal row onto partitions 0..64.
            sel8 = pconst.tile([HPC, HPC * DH], F32, tag="sel8")
            nc.sync.dma_start(sel8[:], sel8_d[:])

            # [65, proj(q=0,k=1), head, S]
            qkT = pqkT.tile([DH, 2, HPC, S], BF16, tag="qkT")
            # v k-tiles with trailing ones column: [128, head, 65+1]
            vh = [pvh.tile([128, HPC, DH + 1], BF16, tag="vh", name=f"vh{j}")
                  for j in range(MT)]
            for j in range(MT):
                nc.gpsimd.memset(vh[j][:, :, DH:DH + 1], 1.0)

            # ---------------- phase 1: projections --------------------------
            with tc.tile_pool(name="px", bufs=9) as px, \
                 tc.tile_pool(name="pw", bufs=9) as pw, \
                 tc.tile_pool(name="pb", bufs=3) as pb, \
                 tc.tile_pool(name="pt1", bufs=6) as pt1:
                for pi, (xd, wd, bd) in [(1, (xk, wk, bk)),
                                         (0, (xq, wq, bq)),
                                         (2, (xv, wv, bv))]:
                    xts, wts = [], []
                    for d in range(8):
                        xt = px.tile([128, S], BF16, tag="x")
                        nc.sync.dma_start(xt[:], xd[d * 128:(d + 1) * 128, :])
                        xts.append(xt)
                        wt = pw.tile([128, HPC, DH], BF16, tag="w")
                        nc.sync.dma_start(wt[:], wd[d * 128:(d + 1) * 128, :])
                        wts.append(wt)
                    bt = pb.tile([1, HPC, DH], BF16, tag="b")
                    nc.sync.dma_start(bt[:], bd[:])

                    for m in range(16):
                        for half in range(2):
                            hs = half * 4
                            ps = psA.tile([128, 4, DH], F32, tag="psA")
                            nc.tensor.matmul(
                                ps[:], ones_col[0:1, :],
                                bt[0:1, hs:hs + 4, :],
                                start=True, stop=False)
                            for d in range(8):
                                nc.tensor.matmul(
                                    ps[:],
                                    xts[d][:, m * 128:(m + 1) * 128],
                                    wts[d][:, hs:hs + 4, :],
                                    start=False, stop=(d == 7))
                            if pi == 2:  # v: straight copy into vh k-tiles
                                nc.vector.tensor_copy(
                                    vh[m][:, hs:hs + 4, 0:DH], ps[:])
                            else:       # q/k: bf16 copy, then PE transpose
                                row = pt1.tile([128, 4, DH], BF16, tag="row")
                                nc.vector.tensor_copy(row[:], ps[:])
                                pstr = psA.tile([128, 4, 128], BF16, tag="psA")
                                for t in range(4):
                                    nc.tensor.transpose(
                                        pstr[0:DH, t, :], row[:, t, :],
                                        ident[:])
                                nc.scalar.copy(
                                    qkT[0:DH, pi, hs:hs + 4,
                                        m * 128:(m + 1) * 128],
                                    pstr[0:DH, :, :])

            # ---------------- phase 2+3: attention + out-proj ---------------
            with tc.tile_pool(name="pm", bufs=1) as pm, \
                 tc.tile_pool(name="pp", bufs=4) as pp, \
                 tc.tile_pool(name="pc", bufs=1) as pc, \
                 tc.tile_pool(name="pwo", bufs=1) as pwo, \
                 tc.tile_pool(name="po", bufs=2) as po, \
                 tc.tile_pool(name="pt2", bufs=4) as pt2:

                # concatT: [dh, head, S]
                ccT = pc.tile([DH, HPC, S], BF16, tag="ccT")
                wots = []
                for h in range(HPC):
                    wot = pwo.tile([DH, D], BF16, tag=f"wo{h}", name=f"wo{h}")
                    nc.sync.dma_start(wot[:], wo[h * DH:(h + 1) * DH, :])
                    wots.append(wot)

                inv_sqrt = 1.0 / math.sqrt(float(DH))
                state = {}   # per-qb tiles for the deferred norm/out-proj

                def attn_head(qb, h, mt, rsall, uovs):
                    ov = psA.tile([128, QW], F32, tag="psA",
                                  name=f"ov{qb}_{h}")
                    for r in range(NR):
                        ss = psS.tile([128, RKT, QW], F32, tag="psS",
                                      name=f"ss{qb}_{h}_{r}")
                        for jj in range(RKT):
                            j = r * RKT + jj
                            nc.tensor.matmul(
                                ss[:, jj, :],
                                qkT[0:DH, 1, h, j * 128:(j + 1) * 128],
                                qkT[0:DH, 0, h, qb * QW:(qb + 1) * QW],
                                start=True, stop=True)
                        pt = pp.tile([128, RKT, QW], BF16, tag="pT",
                                     name=f"pt{qb}_{h}_{r}")
                        nc.scalar.activation(
                            pt[:], ss[:],
                            mybir.ActivationFunctionType.Exp,
                            scale=inv_sqrt)
                        nc.vector.tensor_mul(
                            pt[:], pt[:], mt[:, r * RKT:(r + 1) * RKT, :])
                        for jj in range(RKT):
                            j = r * RKT + jj
                            nc.tensor.matmul(
                                ov[0:DH + 1, :], vh[j][:, h, :],
                                pt[:, jj, :],
                                start=(j == 0), stop=(j == KT - 1))
                    uov = pt2.tile([DH, QW], BF16, tag="uov",
                                   name=f"uov{qb}_{h}", bufs=2 * HPC + 2)
                    nc.vector.tensor_copy(uov[:], ov[0:DH, :])
                    rs2 = pt2.tile([66, QW], F32, tag="rs2",
                                   name=f"rs2_{qb}_{h}", bufs=3)
                    nc.vector.tensor_copy(rs2[64:66, :], ov[64:66, :])
                    nc.gpsimd.dma_start(rsall[h:h + 1, :], rs2[65:66, :])
                    uovs.append(uov)

                def norm_heads(qb, hs):
                    st = state[qb]
                    if "rcall" not in st:
                        rcall = pt2.tile([HPC, QW], F32, tag="rcall",
                                         name=f"rcall{qb}", bufs=2)
                        nc.vector.reciprocal(rcall[:], st["rsall"][:])
                        st["rcall"] = rcall
                    for h in hs:
                        rbp = psA.tile([128, QW], F32, tag="psA",
                                       name=f"rbp{qb}_{h}")
                        nc.tensor.matmul(rbp[0:DH, :],
                                         sel8[:, h * DH:(h + 1) * DH],
                                         st["rcall"][:],
                                         start=True, stop=True)
                        nc.vector.tensor_mul(
                            ccT[0:DH, h, qb * QW:(qb + 1) * QW],
                            rbp[0:DH, :], st["uovs"][h][:])

                def outproj_m(m):
                    osb = po.tile([128, D], F32, tag="osb")
                    for n in range(2):
                        ps = psA.tile([128, QW], F32, tag="psA",
                                      name=f"psop{m}_{n}")
                        for h in range(HPC):
                            nc.tensor.matmul(
                                ps[:],
                                ccT[0:DH, h, m * 128:(m + 1) * 128],
                                wots[h][:, n * QW:(n + 1) * QW],
                                start=(h == 0), stop=(h == HPC - 1))
                        nc.vector.tensor_copy(
                            osb[:, n * QW:(n + 1) * QW], ps[:])
                    nc.gpsimd.dma_start(out[m * 128:(m + 1) * 128, :],
                                        osb[:])

                for qb in range(QB):
                    mt = pm.tile([128, KT, QW], BF16, tag="mask",
                                 name=f"mask{qb}")
                    nc.sync.dma_start(mt[:], mh[qb, :, :])
                    rsall = pt2.tile([HPC, QW], F32, tag="rsall",
                                     name=f"rsall{qb}", bufs=2)
                    uovs = []
                    state[qb] = {"rsall": rsall, "uovs": uovs}
                    for h in range(HPC):
                        attn_head(qb, h, mt, rsall, uovs)
                        # deferred work from the previous q-block, emitted
                        # inside this q-block's ACT-bound slots so the PE
                        # fills its headroom instead of stalling at a tail
                        if qb > 0:
                            if h == 0:
                                norm_heads(qb - 1, range(0, 4))
                            elif h == 1:
                                norm_heads(qb - 1, range(4, HPC))
                            elif 2 <= h <= 5:
                                outproj_m((qb - 1) * 4 + (h - 2))
                # final q-block tail
                norm_heads(QB - 1, range(HPC))
                for m in range((QB - 1) * 4, QB * 4):
                    outproj_m(m)

    return nc


def _prep_inputs(q, k, v, mask, Wq, bqv, Wk, bkv, Wv, bvv, Wo):
    """Per-core input maps (numpy, host-side shard + cast)."""
    in_maps = []
    sel8 = np.zeros((HPC, HPC * DH), np.float32)
    for h in range(HPC):
        sel8[h, h * DH:(h + 1) * DH] = 1.0
    mask_h = {}
    for b in range(B):
        mt = (mask[b, 0] != 0).astype(np.float32).T  # [k, q]
        m4 = mt.reshape(KT, 128, QB, QW).transpose(2, 1, 0, 3)
        mask_h[b] = np.ascontiguousarray(m4.reshape(QB, 128, KT * QW)).astype(BF)
    for c in range(N_CORES):
        b, hh = c // 2, c % 2
        sl = slice(hh * PC, (hh + 1) * PC)
        in_maps.append({
            "xq": np.ascontiguousarray(q[b].T).astype(BF),
            "xk": np.ascontiguousarray(k[b].T).astype(BF),
            "xv": np.ascontiguousarray(v[b].T).astype(BF),
            "maskH": mask_h[b],
            "wqT": np.ascontiguousarray(Wq[sl, :].T).astype(BF),
            "wkT": np.ascontiguousarray(Wk[sl, :].T).astype(BF),
            "wvT": np.ascontiguousarray(Wv[sl, :].T).astype(BF),
            "bq": bqv[sl].reshape(1, PC).astype(BF),
            "bk": bkv[sl].reshape(1, PC).astype(BF),
            "bv": bvv[sl].reshape(1, PC).astype(BF),
            "woT": np.ascontiguousarray(Wo[:, sl].T).astype(BF),
            "sel8": sel8,
        })
    return in_maps


def run_sharded(in_maps, **kwargs):
    if "nc" not in _BUILT:
        _BUILT["nc"] = _build_nc()
    return run_bass_kernel_spmd(_BUILT["nc"], in_maps,
                                core_ids=list(range(N_CORES)), **kwargs)


def kernel(q, k, v, mask, Wq, bq, Wk, bk, Wv, bv, Wo, bo):
    q = np.asarray(q, np.float32)
    k = np.asarray(k, np.float32)
    v = np.asarray(v, np.float32)
    mask = np.asarray(mask)
    in_maps = _prep_inputs(q, k, v, mask,
                           np.asarray(Wq, np.float32), np.asarray(bq, np.float32),
                           np.asarray(Wk, np.float32), np.asarray(bk, np.float32),
                           np.asarray(Wv, np.float32), np.asarray(bv, np.float32),
                           np.asarray(Wo, np.float32))
    res = run_sharded(in_maps)
    bo32 = np.asarray(bo, np.float32)
    out = np.empty((B, S, D), np.float32)
    for b in range(B):
        out[b] = res.results[2 * b]["out"] + res.results[2 * b + 1]["out"] + bo32
    return out
